# revision 1
# baseline (speedup 1.0000x reference)
"""Multi-head attention (Whisper-style, causal) on 8 Trainium2 cores — v2.

Sharding: data-parallel over batch (2) x tensor-parallel over heads (4 groups
of 4 heads).  Core c handles batch c//4, heads [4*(c%4), 4*(c%4)+4).
Host-side prep transposes x / the weight slices and splits them into fp16
hi/lo pairs; host-side combine sums the 4 partials per batch and adds the
bias terms (bv @ Wo.T + bo), which commute past softmax-normalized attention.

v2 structure (vs v1):
  - Engine assignment: phase-2 Activation queue runs exp ONLY; projection
    hi/f32 staging copies run on Act in phase 1 where it is idle; the
    fp16-split lo subtracts and small SBUF scatters run on gpsimd (which
    cannot touch PSUM); PSUM-reading stash/normalize/yt copies run on DVE;
    causal triangular-mask additions are PE matmuls (identity-stationary
    accumulate of the tri tile, ~53ns) instead of DVE tensor_tensor ops.
  - Max-pass reductions (DVE-only work, the phase-1 critical resource) start
    ~13us in: projections are emitted per 1024-column s-chunk with the first
    half of each head-pair's max matmuls interleaved right after s-chunk 0.
  - Small PSUM tiles alternate between two pools (4-deep buffering) so the
    PE never waits in lockstep on a single DVE reduce.
  - The v projection is the phase-1 PE tail filler while DVE drains the last
    heads' reductions.
  - Phase 2 is reordered [q0h0-2, q1h0-2, q0h3, norm0, proj0, q1h3, norm1,
    proj1] so the batched normalization never gates on the slowest head's
    max-pass tail, and the output-projection DMA tail shrinks.
  - DMA emission order feeds the first projection after ~4.5MB instead of
    ~10MB.
"""

import numpy as np

import concourse.bass as bass
import concourse.mybir as mybir
import concourse.tile as tile
from contextlib import ExitStack
from concourse import bacc, bass_utils
from concourse.masks import make_identity, make_causal_mask

B, S, NS, H, DH = 2, 2048, 1024, 16, 64
HPC = 4                 # heads per core
CB = HPC * DH           # 256 projected columns per core
SCALE = DH ** -0.25
NEG = -1e9
NEG16 = -60000.0
P = 128
KSUB = NS // P          # 8 contraction subtiles
NQB = S // P            # 16 q blocks of 128
f32, f16 = mybir.dt.float32, mybir.dt.float16
FX = mybir.ActivationFunctionType

_PROG = None


def build_program(repeat=1, big_bufs=2, pe_bufs=4, qst_bufs=3, ablate=(),
                  debug_dump=False):
    nc = bacc.Bacc("TRN2", target_bir_lowering=False, debug=False)

    xh_d = nc.dram_tensor("xh", [NS, S], f16, kind="ExternalInput").ap()
    xl_d = nc.dram_tensor("xl", [NS, S], f16, kind="ExternalInput").ap()
    wqh_d = nc.dram_tensor("wqh", [NS, CB], f16, kind="ExternalInput").ap()
    wql_d = nc.dram_tensor("wql", [NS, CB], f16, kind="ExternalInput").ap()
    wkh_d = nc.dram_tensor("wkh", [NS, CB], f16, kind="ExternalInput").ap()
    wkl_d = nc.dram_tensor("wkl", [NS, CB], f16, kind="ExternalInput").ap()
    wv_d = nc.dram_tensor("wv", [NS, CB], f16, kind="ExternalInput").ap()
    wo_d = nc.dram_tensor("wo", [CB, NS], f16, kind="ExternalInput").ap()
    sbq_d = nc.dram_tensor("sbq", [CB], f32, kind="ExternalInput").ap()
    sel_d = nc.dram_tensor("sel", [16, 16 * DH], f16,
                           kind="ExternalInput").ap()
    y_d = nc.dram_tensor("y", [S, NS], f16, kind="ExternalOutput").ap()
    if debug_dump:
        dbg_oU = nc.dram_tensor("dbg_oU", [P, 2 * S], f32,
                                kind="ExternalOutput").ap()
        dbg_sums = nc.dram_tensor("dbg_sums", [16, 512], f32,
                                  kind="ExternalOutput").ap()
        dbg_mh = nc.dram_tensor("dbg_mh", [P, 4 * NQB], f32,
                                kind="ExternalOutput").ap()
    y_v = y_d.rearrange("(st p) j -> p st j", p=P)

    with tile.TileContext(nc) as tc, ExitStack() as stack:
        cpool = stack.enter_context(tc.tile_pool(name="cpool", bufs=1))
        wpool = stack.enter_context(tc.tile_pool(name="wpool", bufs=1))
        qkpool = stack.enter_context(tc.tile_pool(name="qkpool", bufs=1))
        # transient SBUF work pools (persist across reps, slots rotate)
        xs = stack.enter_context(tc.tile_pool(name="xs", bufs=1))
        qst_pool = stack.enter_context(tc.tile_pool(name="qst",
                                                    bufs=qst_bufs))
        vstg = stack.enter_context(tc.tile_pool(name="vstg", bufs=5))
        mxw = stack.enter_context(tc.tile_pool(name="mxw", bufs=2))
        scw = stack.enter_context(tc.tile_pool(name="scw", bufs=pe_bufs))
        yw = stack.enter_context(tc.tile_pool(name="yw", bufs=6))
        # PSUM pools: 4 + 2 + 2 banks
        big2 = stack.enter_context(tc.tile_pool(name="big2", bufs=big_bufs,
                                                space="PSUM"))
        po2 = stack.enter_context(tc.tile_pool(name="po2", bufs=2,
                                               space="PSUM"))
        mps2 = stack.enter_context(tc.tile_pool(name="mps2", bufs=2,
                                                space="PSUM"))

        # --- weights + constants: SBUF tiles ---
        ident = cpool.tile([P, P], f32, name="ident")
        ident16 = cpool.tile([P, P], f16, name="ident16")
        tri_std = cpool.tile([P, P], f16, name="tri_std")
        tri_t = cpool.tile([P, P], f16, name="tri_t")   # [k,q]: NEG16 if k>q
        sel = cpool.tile([16, 16 * DH], f16, name="sel")
        wq_hi = wpool.tile([P, KSUB, CB], f16, name="wq_hi")
        wq_lo = wpool.tile([P, KSUB, CB], f16, name="wq_lo")
        wk_hi = wpool.tile([P, KSUB, CB], f16, name="wk_hi")
        wk_lo = wpool.tile([P, KSUB, CB], f16, name="wk_lo")
        wv_t = wpool.tile([P, KSUB, CB], f16, name="wv_t")
        wo_t = wpool.tile([P, 2, NS], f16, name="wo_t")
        sbq_t = wpool.tile([P, 2], f32, name="sbq_t")

        # --- persistent activations ---
        qh2 = [qkpool.tile([P, S], f16, name=f"qh{h}") for h in range(HPC)]
        qla = [qkpool.tile([P, S], f16, name=f"ql{h}") for h in range(HPC)]
        khl = [qkpool.tile([P, S], f16, name=f"kh{h}") for h in range(HPC)]
        kha = [qkpool.tile([P, S], f16, name=f"kl{h}") for h in range(HPC)]
        vt = [qkpool.tile([P, NQB, DH + 1], f16, name=f"v{h}")
              for h in range(HPC)]
        oU = qkpool.tile([P, 2, S], f16, name="oU")     # o.T (norm in place)
        sums = qkpool.tile([16, 512], f32, name="sums")
        rc = qkpool.tile([16, 512], f16, name="rc")
        mh = [qkpool.tile([P, NQB], f32, name=f"m{h}") for h in range(HPC)]

        xh_v = xh_d.rearrange("(ko p) s -> p ko s", p=P)
        xl_v = xl_d.rearrange("(ko p) s -> p ko s", p=P)

        # --- on-chip init (gpsimd/iota; no DMA) ---
        make_identity(nc, ident[:])
        make_identity(nc, ident16[:])
        # f16 causal masks (NEG16 = -60000 is f16-representable and large
        # enough: scores are at most a few thousand in magnitude)
        make_causal_mask(nc, tri_std[:], mask_val=NEG16)  # [q,k]
        nc.gpsimd.memset(tri_t[:], 0.0)
        nc.gpsimd.affine_select(
            out=tri_t[:], in_=tri_t[:],
            compare_op=mybir.AluOpType.is_ge, fill=NEG16, base=0,
            # keep where -x + y >= 0 i.e. q >= k; fill where k > q
            pattern=[[1, P]], channel_multiplier=-1)
        nc.gpsimd.memset(sums[:], 1.0)
        for h in range(HPC):
            nc.gpsimd.memset(kha[h][64:128, :], 0.0)
            nc.gpsimd.memset(qla[h][64:128, :], 0.0)
            nc.gpsimd.memset(kha[h][64:65, :], -1.0)
            nc.gpsimd.memset(vt[h][:, :, DH:DH + 1], 1.0)

        # alternate small PSUM tiles over two pools => 4-deep buffering
        _rr = [0]

        def small_ps(name):
            _rr[0] ^= 1
            pool = (mps2, po2)[_rr[0]]
            return pool.tile([P, 512], f32, name=name,
                             tag=("mps", "po")[_rr[0]])

        def max_chunk(h, qbs, wide=False):
            """Causal row maxima matmuls+reductions for head h, q blocks qbs.

            wide=True routes [128,1024] tiles through big2 (2 banks each):
            2x fewer DVE reductions and 4 matmuls of runway per tile pair.
            """
            for qb in qbs:
                valid_all = qb * P + P
                tile_w = 1024 if wide else 512
                nt = (valid_all + tile_w - 1) // tile_w
                for ti in range(nt):
                    base = ti * tile_w
                    w = min(tile_w, valid_all - base)
                    if wide:
                        ps = big2.tile([P, 1024], f32, name="mwps",
                                       tag="big")
                    else:
                        ps = small_ps("mps")
                    last = ti == nt - 1
                    for sub in range(0, w, 512):
                        sw = min(512, w - sub)
                        diag_here = last and sub + sw == w
                        nc.tensor.matmul(
                            ps[:, sub:sub + sw],
                            qh2[h][:, qb * P:(qb + 1) * P],
                            khl[h][:, base + sub:base + sub + sw],
                            start=True, stop=not diag_here)
                        if diag_here:
                            # diagonal causal mask via PE tri accumulate
                            nc.tensor.matmul(
                                ps[:, w - P:w], ident16[:], tri_std[:],
                                start=False, stop=True,
                                skip_group_check=True)
                    if ti == 0:
                        nc.vector.tensor_reduce(
                            mh[h][:, qb:qb + 1], ps[:, 0:w],
                            axis=mybir.AxisListType.X,
                            op=mybir.AluOpType.max)
                    else:
                        tm = mxw.tile([P, 1], f32, name="tm")
                        nc.vector.tensor_reduce(
                            tm[:], ps[:, 0:w],
                            axis=mybir.AxisListType.X,
                            op=mybir.AluOpType.max)
                        nc.vector.tensor_tensor(
                            mh[h][:, qb:qb + 1], mh[h][:, qb:qb + 1],
                            tm[:], mybir.AluOpType.max)

        def maxb_partA(h):
            # k[0:1024] portion of back q blocks: only needs s-chunk-0 k and
            # s-chunk-1 q, so it runs ~20us before the k s1 projection lands
            for qb in range(8, NQB):
                for sub in range(2):
                    ps = small_ps("mps")
                    nc.tensor.matmul(
                        ps[:, 0:512],
                        qh2[h][:, qb * P:(qb + 1) * P],
                        khl[h][:, sub * 512:sub * 512 + 512],
                        start=True, stop=True)
                    if sub == 0:
                        nc.vector.tensor_reduce(
                            mh[h][:, qb:qb + 1], ps[:, 0:512],
                            axis=mybir.AxisListType.X,
                            op=mybir.AluOpType.max)
                    else:
                        tm = mxw.tile([P, 1], f32, name="tm")
                        nc.vector.tensor_reduce(
                            tm[:], ps[:, 0:512],
                            axis=mybir.AxisListType.X,
                            op=mybir.AluOpType.max)
                        nc.vector.tensor_tensor(
                            mh[h][:, qb:qb + 1], mh[h][:, qb:qb + 1],
                            tm[:], mybir.AluOpType.max)

        def maxb_partB(h, qbs):
            # k[1024:qb*128+128] remainder (diagonal tri mask included)
            for qb in qbs:
                valid = qb * P + P
                w = valid - 1024
                ps = big2.tile([P, 1024], f32, name="mwps", tag="big")
                for sub in range(0, w, 512):
                    sw = min(512, w - sub)
                    diag_here = sub + sw == w
                    nc.tensor.matmul(
                        ps[:, sub:sub + sw],
                        qh2[h][:, qb * P:(qb + 1) * P],
                        khl[h][:, 1024 + sub:1024 + sub + sw],
                        start=True, stop=not diag_here)
                    if diag_here:
                        nc.tensor.matmul(
                            ps[:, w - P:w], ident16[:], tri_std[:],
                            start=False, stop=True, skip_group_check=True)
                tm = mxw.tile([P, 1], f32, name="tm")
                nc.vector.tensor_reduce(
                    tm[:], ps[:, 0:w], axis=mybir.AxisListType.X,
                    op=mybir.AluOpType.max)
                nc.vector.tensor_tensor(
                    mh[h][:, qb:qb + 1], mh[h][:, qb:qb + 1],
                    tm[:], mybir.AluOpType.max)

        def max_finish(h):
            tpm = mps2.tile([NQB, P], f32, name="tpm", tag="mps")
            nc.tensor.transpose(tpm[:], mh[h][:, 0:NQB], ident[:])
            mt = mxw.tile([NQB, P], f16, name="mt")
            nc.scalar.copy(mt[:], tpm[:])
            nc.gpsimd.dma_start(qla[h][64:65, :], mt[:])

        for _rep in range(repeat):
            first = _rep == 0
            # ---- x DMAs (+ weight DMAs threaded in priority order, rep 0) --
            if first:
                nc.sync.dma_start(wq_hi[:],
                                  wqh_d.rearrange("(ko p) c -> p ko c", p=P))
            xhs = [[None] * 4, [None] * 4]
            xls = [[None] * 4, [None] * 4]
            for sc in range(2):
                ss = slice(sc * 1024, (sc + 1) * 1024)
                for pname, dst, view in (("xh", xhs, xh_v), ("xl", xls,
                                                             xl_v)):
                    for piece in range(4):
                        t = xs.tile([P, KSUB // 4, 1024], f16,
                                    name=f"{pname}{piece}{sc}")
                        nc.sync.dma_start(
                            t[:], view[:, piece * 2:piece * 2 + 2, ss])
                        dst[sc][piece] = t
                    if first and sc == 0 and pname == "xh":
                        nc.sync.dma_start(
                            wq_lo[:],
                            wql_d.rearrange("(ko p) c -> p ko c", p=P))
                if first and sc == 0:
                    nc.sync.dma_start(
                        wk_hi[:], wkh_d.rearrange("(ko p) c -> p ko c", p=P))
                    nc.sync.dma_start(
                        wk_lo[:], wkl_d.rearrange("(ko p) c -> p ko c", p=P))
                    nc.sync.dma_start(
                        sbq_t[:], sbq_d.rearrange("(cs p) -> p cs", p=P))
            if first:
                nc.sync.dma_start(wv_t[:],
                                  wv_d.rearrange("(ko p) c -> p ko c", p=P))
                nc.sync.dma_start(wo_t[:],
                                  wo_d.rearrange("(cs p) j -> p cs j", p=P))
                nc.sync.dma_start(sel[:], sel_d[:])

            # ====== Phase 1: QKV projections + interleaved maxima ======
            def proj_qk(cs, proj, sc):
                csl = slice(cs * P, (cs + 1) * P)
                w_hi = wq_hi if proj == "q" else wk_hi
                w_lo = wq_lo if proj == "q" else wk_lo
                xh_c, xl_c = xhs[sc], xls[sc]
                ss = slice(sc * 1024, (sc + 1) * 1024)
                ps = big2.tile([P, 1024], f32, name="qkps", tag="big")
                # pass order (hi,xh),(lo,xh),(hi,xl) tracks the DMA arrival
                # order; ko-outer half-inner reuses each stationary weight
                # subtile for two matmuls (one Ldweights)
                for pi, (wt, xt) in enumerate(
                        ((w_hi, xh_c), (w_lo, xh_c), (w_hi, xl_c))):
                    for ko in range(KSUB):
                        for half in range(2):
                            hsl = slice(half * 512, (half + 1) * 512)
                            nc.tensor.matmul(
                                ps[:, hsl],
                                wt[:, ko, csl],
                                xt[ko // 2][:, ko % 2, hsl],
                                start=(pi == 0 and ko == 0),
                                stop=(pi == 2 and ko == KSUB - 1))
                if proj == "q":
                    nc.scalar.activation(ps[:], ps[:], FX.Identity,
                                         bias=sbq_t[:, cs:cs + 1])
                # dup DMAs gate the max-pass matmuls; subtracts are only
                # needed by phase 2 — keep all dups ahead of the slow
                # gpsimd subtracts in the in-order Pool queue
                subs = []
                for hh in range(2):
                    h = 2 * cs + hh
                    srcp = ps[hh * 64:(hh + 1) * 64, :]
                    st32 = qst_pool.tile([64, 1024], f32, name="st32")
                    if proj == "q":
                        nc.scalar.copy(qh2[h][0:64, ss], srcp)
                        nc.scalar.copy(st32[:], srcp)
                        nc.gpsimd.dma_start(qh2[h][64:128, ss],
                                            qh2[h][0:64, ss])
                        subs.append((qla[h][0:64, ss], st32,
                                     qh2[h][0:64, ss]))
                    else:
                        nc.scalar.copy(khl[h][64:128, ss], srcp)
                        nc.scalar.copy(st32[:], srcp)
                        nc.gpsimd.dma_start(kha[h][0:64, ss],
                                            khl[h][64:128, ss])
                        subs.append((khl[h][0:64, ss], st32,
                                     kha[h][0:64, ss]))
                for out_ap, st32, hi_ap in subs:
                    nc.gpsimd.tensor_tensor(out_ap, st32[:], hi_ap,
                                            mybir.AluOpType.subtract)

            QF = range(0, 8)      # front q blocks (need only s-chunk 0)
            for cs in range(2):
                proj_qk(cs, "q", 0)
                proj_qk(cs, "k", 0)
                if "maxmm" not in ablate:
                    max_chunk(2 * cs, QF)
                    max_chunk(2 * cs + 1, QF)
                proj_qk(cs, "q", 1)
                if "maxmm" not in ablate:
                    maxb_partA(2 * cs)
                    maxb_partA(2 * cs + 1)
                proj_qk(cs, "k", 1)

            def proj_v(st):
                sc, sti = divmod(st, 8)
                psv = mps2.tile([P, 512], f32, name="vps", tag="mps")
                for ko in range(KSUB):
                    nc.tensor.matmul(
                        psv[:, 0:CB],
                        xhs[sc][ko // 2][:, ko % 2, sti * P:(sti + 1) * P],
                        wv_t[:, ko, :],
                        start=(ko == 0), stop=(ko == KSUB - 1))
                vsg = vstg.tile([P, CB], f16, name="vsg")
                nc.scalar.copy(vsg[:], psv[:, 0:CB])
                for h in range(HPC):
                    nc.gpsimd.tensor_copy(
                        vt[h][:, st, 0:DH], vsg[:, h * DH:(h + 1) * DH])

            # h0/h1 back q blocks: DVE-paced maxb qb-pairs (via big2)
            # interleaved 2:1 with independent v-projection tiles so the PE
            # never parks while DVE drains the reductions; cs1's second
            # s-chunk projections then overlap the reduce tail.
            if "maxmm" not in ablate:
                for h in (0, 1):
                    for i, qb0 in enumerate(range(8, NQB, 2)):
                        maxb_partB(h, (qb0, qb0 + 1))
                        proj_v(4 * h + i)
                max_finish(0)
                max_finish(1)

                def maxb_units(h, qb0s):
                    return [lambda h=h, qb0=qb0: maxb_partB(
                        h, (qb0, qb0 + 1)) for qb0 in qb0s]

                def v_units(sts):
                    return [lambda st=st: proj_v(st) for st in sts]
            else:
                for st in range(NQB):
                    proj_v(st)

                def maxb_fill(h, sts):
                    units = []
                    sts = list(sts)
                    for qb0 in range(8, NQB, 2):
                        if sts:
                            units.append(lambda st=sts.pop(0): proj_v(st))
                        units.append(lambda h=h, qb0=qb0: max_chunk(
                            h, (qb0, qb0 + 1), wide=True))
                    units.extend(lambda st=st: proj_v(st) for st in sts)
                    return units

            # ====== Phase 2: scores / exp / pv / norm / proj ======
            def score_head(qc, h, filler=None, fstride=1):
                nkb = (8 * qc + 8) if "scores" not in ablate else 0
                po = [po2.tile([DH + 1, 512], f32,
                               name=f"po{half}", tag="po")
                      for half in range(2)]
                def emit_pv(kb, off, pe):
                    for half in range(2):
                        lo = max(off, half * 512)
                        hi = (half + 1) * 512
                        if lo >= hi:
                            continue
                        stop_kb = (8 * qc + 3) if half == 0 else (nkb - 1)
                        nc.tensor.matmul(
                            po[half][:, lo - half * 512:512],
                            vt[h][:, kb, :], pe[:, lo:hi],
                            start=(kb == 0), stop=(kb == stop_kb))

                # pv is emitted one kb behind scores+exp so the in-order PE
                # queue never parks on an exp: scores(kb+1) runs while
                # exp(kb) is in flight, then pv(kb) is ready.
                pending_pv = None
                for kb in range(nkb):
                    j = max(0, kb - 8 * qc)
                    off = j * P
                    ks = slice(kb * P, (kb + 1) * P)
                    ps = big2.tile([P, 1024], f32, name="sps", tag="big")
                    qbase = qc * 1024
                    regions = [(max(off, r0), r0 + 512,
                                (kb >= 8 * qc) and (r0 <= off < r0 + 512))
                               for r0 in range(off - off % 512, 1024, 512)]
                    for lo, r1, _ in regions:
                        nc.tensor.matmul(
                            ps[:, lo:r1], khl[h][:, ks],
                            qh2[h][:, qbase + lo:qbase + r1],
                            start=True, stop=False)
                    for lo, r1, diag_here in regions:
                        nc.tensor.matmul(
                            ps[:, lo:r1], kha[h][:, ks],
                            qla[h][:, qbase + lo:qbase + r1],
                            start=False, stop=not diag_here)
                        if diag_here:
                            nc.tensor.matmul(
                                ps[:, off:off + P], ident16[:], tri_t[:],
                                start=False, stop=True,
                                skip_group_check=True)
                    if "exp" in ablate:
                        continue
                    pe = scw.tile([P, 1024], f16, name="pe")
                    nc.scalar.activation(pe[:, off:1024],
                                         ps[:, off:1024], FX.Exp)
                    if "pv" in ablate:
                        continue
                    if pending_pv is not None:
                        emit_pv(*pending_pv)
                    if filler and kb % fstride == 0:
                        filler.pop(0)()
                    pending_pv = (kb, off, pe)
                if pending_pv is not None and "pv" not in ablate \
                        and "exp" not in ablate:
                    emit_pv(*pending_pv)
                if "norm" in ablate:
                    return
                # stash unnormalized o and sums (DVE, PSUM reads)
                for half in range(2):
                    q5 = slice((2 * qc + half) * 512,
                               (2 * qc + half + 1) * 512)
                    nc.vector.tensor_copy(
                        oU[(h % 2) * DH:(h % 2 + 1) * DH,
                           h // 2, q5], po[half][0:DH, :])
                    i5 = h * 4 + 2 * qc + half
                    smt = mxw.tile([1, 512], f32, name="smt")
                    nc.vector.tensor_copy(smt[:], po[half][DH:DH + 1, :])
                    nc.gpsimd.dma_start(sums[i5:i5 + 1, :], smt[:])

            def norm_units(qc, halves=(0, 1)):
                units = []

                def recip_unit():
                    with nc.allow_low_precision(
                            reason="1/sums to f16: rel err ~5e-4 "
                                   "well inside tolerance"):
                        nc.vector.reciprocal(rc[:], sums[:])
                units.append(recip_unit)
                for half in halves:
                    for h in range(HPC):
                        def mult_unit(h=h, half=half):
                            i = h * 4 + 2 * qc + half
                            b0 = (h % 2) * DH
                            rbp = mps2.tile([P, 512], f32, name="rbp",
                                            tag="mps")
                            nc.tensor.matmul(rbp[b0:b0 + DH, :],
                                             sel[:, i * DH:(i + 1) * DH],
                                             rc[:], start=True, stop=True)
                            qsl = slice((2 * qc + half) * 512,
                                        (2 * qc + half + 1) * 512)
                            nc.vector.tensor_tensor(
                                oU[b0:b0 + DH, h // 2, qsl],
                                oU[b0:b0 + DH, h // 2, qsl],
                                rbp[b0:b0 + DH, :], mybir.AluOpType.mult)
                        units.append(mult_unit)
                return units

            def proj_units(qc, sts, act_yt=False):
                units = []
                for st in sts:
                    for jc in range(2):
                        def unit(st=st, jc=jc):
                            jsl = slice(jc * 512, (jc + 1) * 512)
                            if qc == 1:
                                ps = small_ps("yp")
                            else:
                                ps = mps2.tile([P, 512], f32, name="yp",
                                               tag="mps")
                            for cs2 in range(2):
                                nc.tensor.matmul(
                                    ps[:], oU[:, cs2, st * P:(st + 1) * P],
                                    wo_t[:, cs2, jsl],
                                    start=(cs2 == 0), stop=(cs2 == 1))
                            yt = yw.tile([P, 512], f16, name="yt")
                            # once exps are done, Act helps drain
                            if act_yt and jc == 0:
                                nc.scalar.copy(yt[:], ps[:])
                            else:
                                nc.vector.tensor_copy(yt[:], ps[:])
                            nc.sync.dma_start(y_v[:, st, jsl], yt[:])
                        units.append(unit)
                return units

            score_head(0, 0)
            score_head(0, 1)
            # h2/h3 back-chunk reductions as fillers in the first two q1
            # heads (maxb pairs first so DVE gets work immediately; v tiles
            # still precede their pv consumers)
            mx = "maxmm" not in ablate
            score_head(1, 0, filler=(maxb_units(2, range(8, NQB, 2))
                                     + v_units(range(8, NQB))) if mx
                       else None)
            if mx:
                max_finish(2)
            score_head(1, 1, filler=maxb_units(3, range(8, NQB, 2)) if mx
                       else None)
            if mx:
                max_finish(3)
            score_head(0, 2)
            score_head(1, 2)
            score_head(0, HPC - 1)
            # normalize+project q-group 0 entirely as fillers inside the
            # last q1 head (dependency-safe: units consumed in order)
            p0_units = (norm_units(0, halves=(0,))
                        + proj_units(0, range(4))
                        + norm_units(0, halves=(1,))[1:]
                        + proj_units(0, range(4, 8))
                        if "norm" not in ablate
                        and "proj" not in ablate else [])
            score_head(1, HPC - 1, filler=p0_units)
            for u in p0_units:
                u()
            if "norm" not in ablate:
                # half-split tail: projection of each q-half starts as soon
                # as that half's normalization lands
                for u in norm_units(1, halves=(0,)):
                    u()
                for u in proj_units(1, range(8, 12)):
                    u()
                for u in norm_units(1, halves=(1,))[1:]:
                    u()
                for u in proj_units(1, range(12, 16)):
                    u()
            if debug_dump:
                nc.gpsimd.dma_start(dbg_oU[:],
                                    oU[:].rearrange("p a b -> p (a b)"))
                nc.sync.dma_start(dbg_sums[:], sums[:])
                for h in range(HPC):
                    nc.sync.dma_start(dbg_mh[:, h * NQB:(h + 1) * NQB],
                                      mh[h][:])

    nc.compile()
    return nc


def _split16(a):
    hi = a.astype(np.float16)
    lo = (a - hi.astype(np.float32)).astype(np.float16)
    return hi, lo


def _prep_core(c, x, Wq, bq, Wk, Wv, Wo):
    b, g = divmod(c, 4)
    cols = slice(g * CB, (g + 1) * CB)
    xT = np.ascontiguousarray(x[b].T).astype(np.float32)
    xh, xl = _split16(xT)
    wq = (SCALE * Wq[cols]).T.astype(np.float32)
    wqh, wql = _split16(wq)
    wk = (SCALE * Wk[cols]).T.astype(np.float32)
    wkh, wkl = _split16(wk)
    wv = Wv[cols].T.astype(np.float16)
    wo = np.ascontiguousarray(Wo[:, cols].T).astype(np.float16)
    sbq = (SCALE * bq[cols]).astype(np.float32)
    sel = np.zeros((16, 16 * DH), np.float16)
    for i in range(16):
        sel[i, i * DH:(i + 1) * DH] = 1.0
    return {"xh": xh, "xl": xl, "wqh": wqh, "wql": wql, "wkh": wkh,
            "wkl": wkl, "wv": np.ascontiguousarray(wv),
            "wo": wo, "sbq": sbq, "sel": sel}


def kernel(x, mask, Wq, bq, Wk, Wv, bv, Wo, bo):
    global _PROG
    if _PROG is None:
        _PROG = build_program()
    x = np.asarray(x, dtype=np.float32)
    in_maps = [_prep_core(c, x, np.asarray(Wq), np.asarray(bq),
                          np.asarray(Wk), np.asarray(Wv), np.asarray(Wo))
               for c in range(8)]
    res = bass_utils.run_bass_kernel_spmd(_PROG, in_maps,
                                          core_ids=list(range(8)))
    host_bias = (np.asarray(bv, np.float32) @ np.asarray(Wo, np.float32).T
                 + np.asarray(bo, np.float32))
    out = np.empty((B, S, NS), np.float32)
    for b in range(B):
        acc = res.results[4 * b]["y"].astype(np.float32)
        for g in range(1, 4):
            acc += res.results[4 * b + g]["y"].astype(np.float32)
        out[b] = acc + host_bias
    return out



# revision 52
# speedup vs baseline: 1.0796x; 1.0796x over previous
"""Multi-head attention (Whisper-style, causal) on 8 Trainium2 cores — v2.

Sharding: data-parallel over batch (2) x tensor-parallel over heads (4 groups
of 4 heads).  Core c handles batch c//4, heads [4*(c%4), 4*(c%4)+4).
Host-side prep transposes x / the weight slices and splits them into fp16
hi/lo pairs; host-side combine sums the 4 partials per batch and adds the
bias terms (bv @ Wo.T + bo), which commute past softmax-normalized attention.

v2 structure (vs v1):
  - Engine assignment: phase-2 Activation queue runs exp ONLY; projection
    hi/f32 staging copies run on Act in phase 1 where it is idle; the
    fp16-split lo subtracts and small SBUF scatters run on gpsimd (which
    cannot touch PSUM); PSUM-reading stash/normalize/yt copies run on DVE;
    causal triangular-mask additions are PE matmuls (identity-stationary
    accumulate of the tri tile, ~53ns) instead of DVE tensor_tensor ops.
  - Max-pass reductions (DVE-only work, the phase-1 critical resource) start
    ~13us in: projections are emitted per 1024-column s-chunk with the first
    half of each head-pair's max matmuls interleaved right after s-chunk 0.
  - Small PSUM tiles alternate between two pools (4-deep buffering) so the
    PE never waits in lockstep on a single DVE reduce.
  - The v projection is the phase-1 PE tail filler while DVE drains the last
    heads' reductions.
  - Phase 2 is reordered [q0h0-2, q1h0-2, q0h3, norm0, proj0, q1h3, norm1,
    proj1] so the batched normalization never gates on the slowest head's
    max-pass tail, and the output-projection DMA tail shrinks.
  - DMA emission order feeds the first projection after ~4.5MB instead of
    ~10MB.
"""

import numpy as np

import concourse.bass as bass
import concourse.mybir as mybir
import concourse.tile as tile
from contextlib import ExitStack
from concourse import bacc, bass_utils
from concourse.masks import make_identity, make_causal_mask

B, S, NS, H, DH = 2, 2048, 1024, 16, 64
HPC = 4                 # heads per core
CB = HPC * DH           # 256 projected columns per core
SCALE = DH ** -0.25
NEG = -1e9
NEG16 = -60000.0
P = 128
KSUB = NS // P          # 8 contraction subtiles
NQB = S // P            # 16 q blocks of 128
f32, f16 = mybir.dt.float32, mybir.dt.float16
f8 = mybir.dt.float8e4
FX = mybir.ActivationFunctionType
DR = mybir.MatmulPerfMode.DoubleRow

_PROG = None


def build_program(repeat=1, big_bufs=2, pe_bufs=5, qst_bufs=3, ablate=(),
                  debug_dump=False):
    nc = bacc.Bacc("TRN2", target_bir_lowering=False, debug=False)

    xh_d = nc.dram_tensor("xh", [NS, S], f16, kind="ExternalInput").ap()
    xl_d = nc.dram_tensor("xl", [NS, S], f16, kind="ExternalInput").ap()
    wqh_d = nc.dram_tensor("wqh", [NS, CB], f16, kind="ExternalInput").ap()
    wql_d = nc.dram_tensor("wql", [NS, CB], f16, kind="ExternalInput").ap()
    wkh_d = nc.dram_tensor("wkh", [NS, CB], f16, kind="ExternalInput").ap()
    wkl_d = nc.dram_tensor("wkl", [NS, CB], f16, kind="ExternalInput").ap()
    wv_d = nc.dram_tensor("wv", [NS, CB], f16, kind="ExternalInput").ap()
    wo_d = nc.dram_tensor("wo", [CB, NS], f16, kind="ExternalInput").ap()
    sbq_d = nc.dram_tensor("sbq", [CB], f32, kind="ExternalInput").ap()
    sel_d = nc.dram_tensor("sel", [16, 16 * DH], f16,
                           kind="ExternalInput").ap()
    y_d = nc.dram_tensor("y", [S, NS], f16, kind="ExternalOutput").ap()
    if debug_dump:
        dbg_oU = nc.dram_tensor("dbg_oU", [P, 2 * S], f32,
                                kind="ExternalOutput").ap()
        dbg_sums = nc.dram_tensor("dbg_sums", [16, 512], f32,
                                  kind="ExternalOutput").ap()
        dbg_mh = nc.dram_tensor("dbg_mh", [P, 4 * NQB], f32,
                                kind="ExternalOutput").ap()
    y_v = y_d.rearrange("(st p) j -> p st j", p=P)

    with tile.TileContext(nc) as tc, ExitStack() as stack:
        cpool = stack.enter_context(tc.tile_pool(name="cpool", bufs=1))
        wpool = stack.enter_context(tc.tile_pool(name="wpool", bufs=1))
        qkpool = stack.enter_context(tc.tile_pool(name="qkpool", bufs=1))
        # transient SBUF work pools (persist across reps, slots rotate)
        xs = stack.enter_context(tc.tile_pool(name="xs", bufs=1))
        qst_pool = stack.enter_context(tc.tile_pool(name="qst",
                                                    bufs=qst_bufs))
        vstg = stack.enter_context(tc.tile_pool(name="vstg", bufs=5))
        mxw = stack.enter_context(tc.tile_pool(name="mxw", bufs=2))
        scw = stack.enter_context(tc.tile_pool(name="scw", bufs=pe_bufs))
        yw = stack.enter_context(tc.tile_pool(name="yw", bufs=6))
        # PSUM pools: 4 + 2 + 2 banks
        big2 = stack.enter_context(tc.tile_pool(name="big2", bufs=big_bufs,
                                                space="PSUM"))
        po2 = stack.enter_context(tc.tile_pool(name="po2", bufs=2,
                                               space="PSUM"))
        mps2 = stack.enter_context(tc.tile_pool(name="mps2", bufs=2,
                                                space="PSUM"))

        # --- weights + constants: SBUF tiles ---
        ident = cpool.tile([P, P], f32, name="ident")
        ident16 = cpool.tile([P, P], f16, name="ident16")
        tri_std = cpool.tile([P, P], f16, name="tri_std")
        tri_t = cpool.tile([P, P], f16, name="tri_t")   # [k,q]: NEG16 if k>q
        sel = cpool.tile([16, 16 * DH], f16, name="sel")
        wq_hi = wpool.tile([P, KSUB, CB], f16, name="wq_hi")
        wq_lo = wpool.tile([P, KSUB, CB], f16, name="wq_lo")
        wk_hi = wpool.tile([P, KSUB, CB], f16, name="wk_hi")
        wk_lo = wpool.tile([P, KSUB, CB], f16, name="wk_lo")
        wv_t = wpool.tile([P, KSUB, CB], f16, name="wv_t")
        wo_t = wpool.tile([P, 2, NS], f16, name="wo_t")
        sbq_t = wpool.tile([P, 2], f32, name="sbq_t")

        # --- persistent activations ---
        # q65[h]: rows 0:64 = qA (f16 hi of scaled q), row 64 = m (row max)
        # k65[h]: rows 0:64 = kA, row 64 = -1  => hi matmul yields kA.qA - m
        # q8/k8 (head-pair packed, rows h%2*64): DoubleRow fp8 correction
        #   q8 slots: (qB*64, qA/64); k8 slots: (kA/64, kB*64)
        #   => DR(k8, q8) = kA.qB + kB.qA (the f16-rounding correction)
        q65 = [qkpool.tile([65, S], f16, name=f"q65{h}") for h in range(HPC)]
        k65 = [qkpool.tile([65, S], f16, name=f"k65{h}") for h in range(HPC)]
        q8 = [qkpool.tile([P, 2, S], f8, name=f"q8{hp}") for hp in range(2)]
        k8 = [qkpool.tile([P, 2, S], f8, name=f"k8{hp}") for hp in range(2)]
        vt = [qkpool.tile([P, NQB, DH + 1], f16, name=f"v{h}")
              for h in range(HPC)]
        oU = qkpool.tile([P, 2, S], f16, name="oU")     # o.T (norm in place)
        sums = qkpool.tile([16, 512], f32, name="sums")
        rc = qkpool.tile([16, 512], f16, name="rc")
        mh = [qkpool.tile([P, NQB], f32, name=f"m{h}") for h in range(HPC)]

        xh_v = xh_d.rearrange("(ko p) s -> p ko s", p=P)
        xl_v = xl_d.rearrange("(ko p) s -> p ko s", p=P)

        # --- on-chip init (gpsimd/iota; no DMA) ---
        make_identity(nc, ident[:])
        make_identity(nc, ident16[:])
        # f16 causal masks (NEG16 = -60000 is f16-representable and large
        # enough: scores are at most a few thousand in magnitude)
        make_causal_mask(nc, tri_std[:], mask_val=NEG16)  # [q,k]
        nc.gpsimd.memset(tri_t[:], 0.0)
        nc.gpsimd.affine_select(
            out=tri_t[:], in_=tri_t[:],
            compare_op=mybir.AluOpType.is_ge, fill=NEG16, base=0,
            # keep where -x + y >= 0 i.e. q >= k; fill where k > q
            pattern=[[1, P]], channel_multiplier=-1)
        nc.gpsimd.memset(sums[:], 1.0)
        for h in range(HPC):
            nc.gpsimd.memset(k65[h][64:65, :], -1.0)
            nc.gpsimd.memset(vt[h][:, :, DH:DH + 1], 1.0)

        # alternate small PSUM tiles over two pools => 4-deep buffering
        _rr = [0]

        def small_ps(name):
            _rr[0] ^= 1
            pool = (mps2, po2)[_rr[0]]
            return pool.tile([P, 512], f32, name=name,
                             tag=("mps", "po")[_rr[0]])

        def max_chunk(h, qbs, wide=False):
            """Causal row maxima matmuls+reductions for head h, q blocks qbs.

            wide=True routes [128,1024] tiles through big2 (2 banks each):
            2x fewer DVE reductions and 4 matmuls of runway per tile pair.
            """
            for qb in qbs:
                valid_all = qb * P + P
                tile_w = 1024 if wide else 512
                nt = (valid_all + tile_w - 1) // tile_w
                for ti in range(nt):
                    base = ti * tile_w
                    w = min(tile_w, valid_all - base)
                    if wide:
                        ps = big2.tile([P, 1024], f32, name="mwps",
                                       tag="big")
                    else:
                        ps = small_ps("mps")
                    last = ti == nt - 1
                    for sub in range(0, w, 512):
                        sw = min(512, w - sub)
                        diag_here = last and sub + sw == w
                        nc.tensor.matmul(
                            ps[:, sub:sub + sw],
                            q65[h][0:64, qb * P:(qb + 1) * P],
                            k65[h][0:64, base + sub:base + sub + sw],
                            start=True, stop=not diag_here)
                        if diag_here:
                            # diagonal causal mask via PE tri accumulate
                            nc.tensor.matmul(
                                ps[:, w - P:w], ident16[:], tri_std[:],
                                start=False, stop=True,
                                skip_group_check=True)
                    if ti == 0:
                        nc.vector.tensor_reduce(
                            mh[h][:, qb:qb + 1], ps[:, 0:w],
                            axis=mybir.AxisListType.X,
                            op=mybir.AluOpType.max)
                    else:
                        tm = mxw.tile([P, 1], f32, name="tm")
                        nc.vector.tensor_reduce(
                            tm[:], ps[:, 0:w],
                            axis=mybir.AxisListType.X,
                            op=mybir.AluOpType.max)
                        nc.vector.tensor_tensor(
                            mh[h][:, qb:qb + 1], mh[h][:, qb:qb + 1],
                            tm[:], mybir.AluOpType.max)

        def maxb_partA(h, qbs=range(8, NQB)):
            # k[0:1024] portion of back q blocks: only needs s-chunk-0 k and
            # s-chunk-1 q, so it runs ~20us before the k s1 projection lands
            for qb in qbs:
                for sub in range(2):
                    ps = small_ps("mps")
                    nc.tensor.matmul(
                        ps[:, 0:512],
                        q65[h][0:64, qb * P:(qb + 1) * P],
                        k65[h][0:64, sub * 512:sub * 512 + 512],
                        start=True, stop=True)
                    if sub == 0:
                        nc.vector.tensor_reduce(
                            mh[h][:, qb:qb + 1], ps[:, 0:512],
                            axis=mybir.AxisListType.X,
                            op=mybir.AluOpType.max)
                    else:
                        tm = mxw.tile([P, 1], f32, name="tm")
                        nc.vector.tensor_reduce(
                            tm[:], ps[:, 0:512],
                            axis=mybir.AxisListType.X,
                            op=mybir.AluOpType.max)
                        nc.vector.tensor_tensor(
                            mh[h][:, qb:qb + 1], mh[h][:, qb:qb + 1],
                            tm[:], mybir.AluOpType.max)

        def maxb_partB(h, qbs):
            # k[1024:qb*128+128] remainder (diagonal tri mask included)
            for qb in qbs:
                valid = qb * P + P
                w = valid - 1024
                ps = big2.tile([P, 1024], f32, name="mwps", tag="big")
                for sub in range(0, w, 512):
                    sw = min(512, w - sub)
                    diag_here = sub + sw == w
                    nc.tensor.matmul(
                        ps[:, sub:sub + sw],
                        q65[h][0:64, qb * P:(qb + 1) * P],
                        k65[h][0:64, 1024 + sub:1024 + sub + sw],
                        start=True, stop=not diag_here)
                    if diag_here:
                        nc.tensor.matmul(
                            ps[:, w - P:w], ident16[:], tri_std[:],
                            start=False, stop=True, skip_group_check=True)
                tm = mxw.tile([P, 1], f32, name="tm")
                nc.vector.tensor_reduce(
                    tm[:], ps[:, 0:w], axis=mybir.AxisListType.X,
                    op=mybir.AluOpType.max)
                nc.vector.tensor_tensor(
                    mh[h][:, qb:qb + 1], mh[h][:, qb:qb + 1],
                    tm[:], mybir.AluOpType.max)

        def max_finish(h):
            tpm = mps2.tile([NQB, P], f32, name="tpm", tag="mps")
            nc.tensor.transpose(tpm[:], mh[h][:, 0:NQB], ident[:])
            mt = mxw.tile([NQB, P], f16, name="mt")
            nc.scalar.copy(mt[:], tpm[:])
            nc.gpsimd.dma_start(q65[h][64:65, :], mt[:])

        for _rep in range(repeat):
            first = _rep == 0
            # ---- x DMAs (+ weight DMAs threaded in priority order, rep 0).
            # cs0 weight halves lead so the first projection starts ~1.7us
            # in; cs1 halves + wv/wo trail the sc1 x pieces. ----
            wv_q = wqh_d.rearrange("(ko p) c -> p ko c", p=P)
            wv_ql = wql_d.rearrange("(ko p) c -> p ko c", p=P)
            wv_k = wkh_d.rearrange("(ko p) c -> p ko c", p=P)
            wv_kl = wkl_d.rearrange("(ko p) c -> p ko c", p=P)
            if first:
                nc.sync.dma_start(wq_hi[:, :, 0:P], wv_q[:, :, 0:P])
            xhs = [[None] * 4, [None] * 4]
            xls = [[None] * 4, [None] * 4]
            for sc in range(2):
                ss = slice(sc * 1024, (sc + 1) * 1024)
                for pname, dst, view in (("xh", xhs, xh_v), ("xl", xls,
                                                             xl_v)):
                    for piece in range(4):
                        t = xs.tile([P, KSUB // 4, 1024], f16,
                                    name=f"{pname}{piece}{sc}")
                        nc.sync.dma_start(
                            t[:], view[:, piece * 2:piece * 2 + 2, ss])
                        dst[sc][piece] = t
                        if first and sc == 0 and pname == "xh" \
                                and piece == 0:
                            nc.sync.dma_start(wq_lo[:, :, 0:P],
                                              wv_ql[:, :, 0:P])
                if first and sc == 0:
                    nc.sync.dma_start(
                        sbq_t[:], sbq_d.rearrange("(cs p) -> p cs", p=P))
                    nc.sync.dma_start(wk_hi[:, :, 0:P], wv_k[:, :, 0:P])
                    nc.sync.dma_start(wk_lo[:, :, 0:P], wv_kl[:, :, 0:P])
            if first:
                nc.sync.dma_start(wq_hi[:, :, P:CB], wv_q[:, :, P:CB])
                nc.sync.dma_start(wq_lo[:, :, P:CB], wv_ql[:, :, P:CB])
                nc.sync.dma_start(wk_hi[:, :, P:CB], wv_k[:, :, P:CB])
                nc.sync.dma_start(wk_lo[:, :, P:CB], wv_kl[:, :, P:CB])
                nc.sync.dma_start(wv_t[:],
                                  wv_d.rearrange("(ko p) c -> p ko c", p=P))
                nc.sync.dma_start(wo_t[:],
                                  wo_d.rearrange("(cs p) j -> p cs j", p=P))
                nc.sync.dma_start(sel[:], sel_d[:])

            # ====== Phase 1: QKV projections + interleaved maxima ======
            def proj_qk(cs, proj, sc):
                csl = slice(cs * P, (cs + 1) * P)
                w_hi = wq_hi if proj == "q" else wk_hi
                w_lo = wq_lo if proj == "q" else wk_lo
                xh_c, xl_c = xhs[sc], xls[sc]
                ss = slice(sc * 1024, (sc + 1) * 1024)
                ps = big2.tile([P, 1024], f32, name="qkps", tag="big")
                # pass order (hi,xh),(lo,xh),(hi,xl) tracks the DMA arrival
                # order; ko-outer half-inner reuses each stationary weight
                # subtile for two matmuls (one Ldweights)
                for pi, (wt, xt) in enumerate(
                        ((w_hi, xh_c), (w_lo, xh_c), (w_hi, xl_c))):
                    for ko in range(KSUB):
                        for half in range(2):
                            hsl = slice(half * 512, (half + 1) * 512)
                            nc.tensor.matmul(
                                ps[:, hsl],
                                wt[:, ko, csl],
                                xt[ko // 2][:, ko % 2, hsl],
                                start=(pi == 0 and ko == 0),
                                stop=(pi == 2 and ko == KSUB - 1))
                if proj == "q":
                    nc.scalar.activation(ps[:], ps[:], FX.Identity,
                                         bias=sbq_t[:, cs:cs + 1])
                # staging: Act hi-copy + f32 snapshot; Pool residual
                # subtract (f16) then fp8 slot conversions for DoubleRow
                At = q65 if proj == "q" else k65
                T8 = q8[cs] if proj == "q" else k8[cs]
                qBt = qst_pool.tile([P, 1024], f16, name="qBt")
                subs = []
                for hh in range(2):
                    h = 2 * cs + hh
                    rsl = slice(hh * 64, (hh + 1) * 64)
                    srcp = ps[rsl, :]
                    st32 = qst_pool.tile([64, 1024], f32, name="st32")
                    nc.scalar.copy(At[h][0:64, ss], srcp)
                    nc.scalar.copy(st32[:], srcp)
                    subs.append((qBt[rsl, :], st32, At[h][0:64, ss]))
                for out_ap, st32, hi_ap in subs:
                    nc.gpsimd.tensor_tensor(out_ap, st32[:], hi_ap,
                                            mybir.AluOpType.subtract)
                # fp8 slots: lo slot is the residual*64, hi slot is hi/64
                lo_slot, hi_slot = (0, 1) if proj == "q" else (1, 0)
                nc.gpsimd.tensor_scalar_mul(T8[:, lo_slot, ss], qBt[:], 64.0)
                for hh in range(2):
                    h = 2 * cs + hh
                    rsl = slice(hh * 64, (hh + 1) * 64)
                    nc.gpsimd.tensor_scalar_mul(T8[rsl, hi_slot, ss],
                                                At[h][0:64, ss], 1.0 / 64.0)

            QF = range(0, 8)      # front q blocks (need only s-chunk 0)
            for cs in range(2):
                proj_qk(cs, "q", 0)
                proj_qk(cs, "k", 0)
                if "maxmm" not in ablate:
                    max_chunk(2 * cs, QF)
                    max_chunk(2 * cs + 1, QF)
                proj_qk(cs, "q", 1)
                if "maxmm" not in ablate:
                    maxb_partA(2 * cs)
                    maxb_partA(2 * cs + 1)
                proj_qk(cs, "k", 1)

            def proj_v(st, dve_stage=False):
                sc, sti = divmod(st, 8)
                psv = mps2.tile([P, 512], f32, name="vps", tag="mps")
                for ko in range(KSUB):
                    nc.tensor.matmul(
                        psv[:, 0:CB],
                        xhs[sc][ko // 2][:, ko % 2, sti * P:(sti + 1) * P],
                        wv_t[:, ko, :],
                        start=(ko == 0), stop=(ko == KSUB - 1))
                vsg = vstg.tile([P, CB], f16, name="vsg")
                # as a phase-2 filler, stage on DVE so the Act queue stays
                # clear for the exp chain
                if dve_stage:
                    nc.vector.tensor_copy(vsg[:], psv[:, 0:CB])
                else:
                    nc.scalar.copy(vsg[:], psv[:, 0:CB])
                for h in range(HPC):
                    nc.gpsimd.tensor_copy(
                        vt[h][:, st, 0:DH], vsg[:, h * DH:(h + 1) * DH])

            # h0/h1 back q blocks: DVE-paced maxb qb-pairs (via big2)
            # interleaved 1:1 with independent v-projection tiles so the PE
            # never parks while DVE drains the reductions
            if "maxmm" not in ablate:
                for h in (0, 1):
                    for i, qb0 in enumerate(range(8, NQB, 2)):
                        maxb_partB(h, (qb0, qb0 + 1))
                        proj_v(4 * h + i)
                max_finish(0)
                max_finish(1)
            else:
                for st in range(8):
                    proj_v(st)

            def maxb_units(h, qb0s):
                return [lambda h=h, qb0=qb0: maxb_partB(
                    h, (qb0, qb0 + 1)) for qb0 in qb0s]

            def v_units(sts):
                return [lambda st=st: proj_v(st, dve_stage=True)
                        for st in sts]

            def _maxA_wide(h, qb):
                # big2-based partA variant, safe as a score_head filler
                # (po2/mps2 untouched)
                ps = big2.tile([P, 1024], f32, name="mwps", tag="big")
                for sub in range(2):
                    nc.tensor.matmul(
                        ps[:, sub * 512:sub * 512 + 512],
                        q65[h][0:64, qb * P:(qb + 1) * P],
                        k65[h][0:64, sub * 512:sub * 512 + 512],
                        start=True, stop=True)
                nc.vector.tensor_reduce(
                    mh[h][:, qb:qb + 1], ps[:],
                    axis=mybir.AxisListType.X, op=mybir.AluOpType.max)

            def maxA_units(h):
                return [lambda h=h, qb=qb: _maxA_wide(h, qb)
                        for qb in range(8, NQB)]

            # ====== Phase 2: scores / exp / pv / norm / proj ======
            def score_head(qc, h, filler=None, fstride=1):
                nkb = (8 * qc + 8) if "scores" not in ablate else 0
                po = [po2.tile([DH + 1, 512], f32,
                               name=f"po{half}", tag="po")
                      for half in range(2)]
                def emit_pv(kb, off, pe):
                    for half in range(2):
                        lo = max(off, half * 512)
                        hi = (half + 1) * 512
                        if lo >= hi:
                            continue
                        stop_kb = (8 * qc + 3) if half == 0 else (nkb - 1)
                        nc.tensor.matmul(
                            po[half][:, lo - half * 512:512],
                            vt[h][:, kb, :], pe[:, lo:hi],
                            start=(kb == 0), stop=(kb == stop_kb))

                # pv is emitted two kb behind scores+exp so the in-order PE
                # queue never parks on an exp: scores(kb+1..2) run while
                # exp(kb) is in flight, then pv(kb) is ready.
                from collections import deque as _dq
                pending_pv = _dq()
                for kb in range(nkb):
                    j = max(0, kb - 8 * qc)
                    off = j * P
                    ks = slice(kb * P, (kb + 1) * P)
                    ps = big2.tile([P, 1024], f32, name="sps", tag="big")
                    qbase = qc * 1024
                    regions = [(max(off, r0), r0 + 512,
                                (kb >= 8 * qc) and (r0 <= off < r0 + 512))
                               for r0 in range(off - off % 512, 1024, 512)]
                    hp, hr = h // 2, (h % 2) * 64
                    for lo, r1, _ in regions:
                        # 65-contraction: kA.qA - m (the -1/m rows ride)
                        nc.tensor.matmul(
                            ps[:, lo:r1], k65[h][:, ks],
                            q65[h][:, qbase + lo:qbase + r1],
                            start=True, stop=False)
                    for lo, r1, diag_here in regions:
                        # fp8 DoubleRow correction: kA.qB + kB.qA
                        for c0 in range(lo, r1, 256):
                            c1 = min(c0 + 256, r1)
                            nc.tensor.matmul(
                                ps[:, c0:c1],
                                k8[hp][hr:hr + 64, :, ks],
                                q8[hp][hr:hr + 64, :,
                                       qbase + c0:qbase + c1],
                                start=False,
                                stop=(c1 == r1 and not diag_here),
                                perf_mode=DR)
                        if diag_here:
                            nc.tensor.matmul(
                                ps[:, off:off + P], ident16[:], tri_t[:],
                                start=False, stop=True,
                                skip_group_check=True)
                    if "exp" in ablate:
                        continue
                    pe = scw.tile([P, 1024], f16, name="pe")
                    nc.scalar.activation(pe[:, off:1024],
                                         ps[:, off:1024], FX.Exp)
                    if "pv" in ablate:
                        continue
                    if len(pending_pv) >= 2:
                        emit_pv(*pending_pv.popleft())
                    if filler and kb % fstride == 0:
                        filler.pop(0)()
                        if len(filler) > nkb - kb - 1:
                            filler.pop(0)()
                    pending_pv.append((kb, off, pe))
                if "pv" not in ablate and "exp" not in ablate:
                    while pending_pv:
                        emit_pv(*pending_pv.popleft())
                if "norm" in ablate:
                    return
                # stash unnormalized o and sums (DVE, PSUM reads)
                for half in range(2):
                    q5 = slice((2 * qc + half) * 512,
                               (2 * qc + half + 1) * 512)
                    nc.vector.tensor_copy(
                        oU[(h % 2) * DH:(h % 2 + 1) * DH,
                           h // 2, q5], po[half][0:DH, :])
                    i5 = h * 4 + 2 * qc + half
                    smt = mxw.tile([1, 512], f32, name="smt")
                    nc.vector.tensor_copy(smt[:], po[half][DH:DH + 1, :])
                    nc.gpsimd.dma_start(sums[i5:i5 + 1, :], smt[:])

            def norm_units(qc, halves=(0, 1)):
                units = []

                def recip_unit():
                    with nc.allow_low_precision(
                            reason="1/sums to f16: rel err ~5e-4 "
                                   "well inside tolerance"):
                        nc.vector.reciprocal(rc[:], sums[:])
                units.append(recip_unit)
                for half in halves:
                    for h in range(HPC):
                        def mult_unit(h=h, half=half):
                            i = h * 4 + 2 * qc + half
                            b0 = (h % 2) * DH
                            rbp = mps2.tile([P, 512], f32, name="rbp",
                                            tag="mps")
                            nc.tensor.matmul(rbp[b0:b0 + DH, :],
                                             sel[:, i * DH:(i + 1) * DH],
                                             rc[:], start=True, stop=True)
                            qsl = slice((2 * qc + half) * 512,
                                        (2 * qc + half + 1) * 512)
                            nc.vector.tensor_tensor(
                                oU[b0:b0 + DH, h // 2, qsl],
                                oU[b0:b0 + DH, h // 2, qsl],
                                rbp[b0:b0 + DH, :], mybir.AluOpType.mult)
                        units.append(mult_unit)
                return units

            def proj_units(qc, sts, act_yt=False):
                units = []
                for st in sts:
                    for jc in range(2):
                        def unit(st=st, jc=jc):
                            jsl = slice(jc * 512, (jc + 1) * 512)
                            if qc == 1:
                                ps = small_ps("yp")
                            else:
                                ps = mps2.tile([P, 512], f32, name="yp",
                                               tag="mps")
                            for cs2 in range(2):
                                nc.tensor.matmul(
                                    ps[:], oU[:, cs2, st * P:(st + 1) * P],
                                    wo_t[:, cs2, jsl],
                                    start=(cs2 == 0), stop=(cs2 == 1))
                            yt = yw.tile([P, 512], f16, name="yt")
                            # once exps are done, Act drains the psum reads
                            if act_yt:
                                nc.scalar.copy(yt[:], ps[:])
                            else:
                                nc.vector.tensor_copy(yt[:], ps[:])
                            nc.sync.dma_start(y_v[:, st, jsl], yt[:])
                        units.append(unit)
                return units

            # ---- Phase B: h0/h1 scores+pv overlapped with the cs1
            # projections, the whole h2/h3 max pass, and v st8..15 ----
            mx = "maxmm" not in ablate
            score_head(0, 0)
            score_head(0, 1)
            score_head(1, 0, filler=(maxb_units(2, range(8, NQB, 2))
                                     + v_units(range(8, NQB))) if mx
                       else None)
            if mx:
                max_finish(2)
            score_head(1, 1, filler=maxb_units(3, range(8, NQB, 2)) if mx
                       else None)
            if mx:
                max_finish(3)
            # ---- Phase C: h2/h3 scores+pv, norm/proj q-group 0 fillers ----
            score_head(0, 2)
            score_head(1, 2)
            score_head(0, HPC - 1)
            # normalize+project q-group 0 entirely as fillers inside the
            # last q1 head (dependency-safe: units consumed in order)
            p0_units = (norm_units(0, halves=(0,))
                        + proj_units(0, range(4))
                        + norm_units(0, halves=(1,))[1:]
                        + proj_units(0, range(4, 8))
                        if "norm" not in ablate
                        and "proj" not in ablate else [])
            score_head(1, HPC - 1, filler=p0_units)
            for u in p0_units:
                u()
            if "norm" not in ablate:
                # half-split tail: projection of each q-half starts as soon
                # as that half's normalization lands
                for u in norm_units(1, halves=(0,)):
                    u()
                for u in proj_units(1, range(8, 12), act_yt=True):
                    u()
                for u in norm_units(1, halves=(1,))[1:]:
                    u()
                for u in proj_units(1, range(12, 16), act_yt=True):
                    u()
            if debug_dump:
                nc.gpsimd.dma_start(dbg_oU[:],
                                    oU[:].rearrange("p a b -> p (a b)"))
                nc.sync.dma_start(dbg_sums[:], sums[:])
                for h in range(HPC):
                    nc.sync.dma_start(dbg_mh[:, h * NQB:(h + 1) * NQB],
                                      mh[h][:])

    nc.compile()
    return nc


def _split16(a):
    hi = a.astype(np.float16)
    lo = (a - hi.astype(np.float32)).astype(np.float16)
    return hi, lo


def _prep_core(c, x, Wq, bq, Wk, Wv, Wo):
    b, g = divmod(c, 4)
    cols = slice(g * CB, (g + 1) * CB)
    xT = np.ascontiguousarray(x[b].T).astype(np.float32)
    xh, xl = _split16(xT)
    wq = (SCALE * Wq[cols]).T.astype(np.float32)
    wqh, wql = _split16(wq)
    wk = (SCALE * Wk[cols]).T.astype(np.float32)
    wkh, wkl = _split16(wk)
    wv = Wv[cols].T.astype(np.float16)
    wo = np.ascontiguousarray(Wo[:, cols].T).astype(np.float16)
    sbq = (SCALE * bq[cols]).astype(np.float32)
    sel = np.zeros((16, 16 * DH), np.float16)
    for i in range(16):
        sel[i, i * DH:(i + 1) * DH] = 1.0
    return {"xh": xh, "xl": xl, "wqh": wqh, "wql": wql, "wkh": wkh,
            "wkl": wkl, "wv": np.ascontiguousarray(wv),
            "wo": wo, "sbq": sbq, "sel": sel}


def kernel(x, mask, Wq, bq, Wk, Wv, bv, Wo, bo):
    global _PROG
    if _PROG is None:
        _PROG = build_program()
    x = np.asarray(x, dtype=np.float32)
    in_maps = [_prep_core(c, x, np.asarray(Wq), np.asarray(bq),
                          np.asarray(Wk), np.asarray(Wv), np.asarray(Wo))
               for c in range(8)]
    res = bass_utils.run_bass_kernel_spmd(_PROG, in_maps,
                                          core_ids=list(range(8)))
    host_bias = (np.asarray(bv, np.float32) @ np.asarray(Wo, np.float32).T
                 + np.asarray(bo, np.float32))
    out = np.empty((B, S, NS), np.float32)
    for b in range(B):
        acc = res.results[4 * b]["y"].astype(np.float32)
        for g in range(1, 4):
            acc += res.results[4 * b + g]["y"].astype(np.float32)
        out[b] = acc + host_bias
    return out



# revision 71
# speedup vs baseline: 1.2326x; 1.1417x over previous
"""Multi-head attention (Whisper-style, causal) on 8 Trainium2 cores — v2.

Sharding: data-parallel over batch (2) x tensor-parallel over heads (4 groups
of 4 heads).  Core c handles batch c//4, heads [4*(c%4), 4*(c%4)+4).
Host-side prep transposes x / the weight slices and splits them into fp16
hi/lo pairs; host-side combine sums the 4 partials per batch and adds the
bias terms (bv @ Wo.T + bo), which commute past softmax-normalized attention.

v2 structure (vs v1):
  - Engine assignment: phase-2 Activation queue runs exp ONLY; projection
    hi/f32 staging copies run on Act in phase 1 where it is idle; the
    fp16-split lo subtracts and small SBUF scatters run on gpsimd (which
    cannot touch PSUM); PSUM-reading stash/normalize/yt copies run on DVE;
    causal triangular-mask additions are PE matmuls (identity-stationary
    accumulate of the tri tile, ~53ns) instead of DVE tensor_tensor ops.
  - Max-pass reductions (DVE-only work, the phase-1 critical resource) start
    ~13us in: projections are emitted per 1024-column s-chunk with the first
    half of each head-pair's max matmuls interleaved right after s-chunk 0.
  - Small PSUM tiles alternate between two pools (4-deep buffering) so the
    PE never waits in lockstep on a single DVE reduce.
  - The v projection is the phase-1 PE tail filler while DVE drains the last
    heads' reductions.
  - Phase 2 is reordered [q0h0-2, q1h0-2, q0h3, norm0, proj0, q1h3, norm1,
    proj1] so the batched normalization never gates on the slowest head's
    max-pass tail, and the output-projection DMA tail shrinks.
  - DMA emission order feeds the first projection after ~4.5MB instead of
    ~10MB.
"""

import numpy as np

import concourse.bass as bass
import concourse.mybir as mybir
import concourse.tile as tile
from contextlib import ExitStack
from concourse import bacc, bass_utils
from concourse.masks import make_identity, make_causal_mask

B, S, NS, H, DH = 2, 2048, 1024, 16, 64
HPC = 4                 # heads per core
CB = HPC * DH           # 256 projected columns per core
SCALE = DH ** -0.25
NEG = -1e9
NEG16 = -60000.0
P = 128
KSUB = NS // P          # 8 contraction subtiles
NQB = S // P            # 16 q blocks of 128
f32, f16 = mybir.dt.float32, mybir.dt.float16
f8 = mybir.dt.float8e4
f8e5 = mybir.dt.float8e5
FX = mybir.ActivationFunctionType
DR = mybir.MatmulPerfMode.DoubleRow

_PROG = None


def build_program(repeat=1, big_bufs=2, pe_bufs=12, qst_bufs=2, ablate=(),
                  debug_dump=False):
    nc = bacc.Bacc("TRN2", target_bir_lowering=False, debug=False)

    xh_d = nc.dram_tensor("xh", [NS, S], f16, kind="ExternalInput").ap()
    xp8_d = nc.dram_tensor("xp8", [NS, 2, S], f8e5, kind="ExternalInput").ap()
    wqh_d = nc.dram_tensor("wqh", [NS, CB], f16, kind="ExternalInput").ap()
    wq8_d = nc.dram_tensor("wq8", [NS, 2, CB], f8e5,
                           kind="ExternalInput").ap()
    wkh_d = nc.dram_tensor("wkh", [NS, CB], f16, kind="ExternalInput").ap()
    wk8_d = nc.dram_tensor("wk8", [NS, 2, CB], f8e5,
                           kind="ExternalInput").ap()
    wv_d = nc.dram_tensor("wv", [NS, CB], f16, kind="ExternalInput").ap()
    wo_d = nc.dram_tensor("wo", [CB, NS], f16, kind="ExternalInput").ap()
    sbq_d = nc.dram_tensor("sbq", [CB], f32, kind="ExternalInput").ap()
    sel_d = nc.dram_tensor("sel", [16, 16 * DH], f16,
                           kind="ExternalInput").ap()
    y_d = nc.dram_tensor("y", [S, NS], f16, kind="ExternalOutput").ap()
    if debug_dump:
        dbg_oU = nc.dram_tensor("dbg_oU", [P, 2 * S], f32,
                                kind="ExternalOutput").ap()
        dbg_sums = nc.dram_tensor("dbg_sums", [16, 512], f32,
                                  kind="ExternalOutput").ap()
        dbg_mh = nc.dram_tensor("dbg_mh", [P, 4 * NQB], f32,
                                kind="ExternalOutput").ap()
    y_v = y_d.rearrange("(st p) j -> p st j", p=P)

    with tile.TileContext(nc) as tc, ExitStack() as stack:
        cpool = stack.enter_context(tc.tile_pool(name="cpool", bufs=1))
        wpool = stack.enter_context(tc.tile_pool(name="wpool", bufs=1))
        qkpool = stack.enter_context(tc.tile_pool(name="qkpool", bufs=1))
        # transient SBUF work pools (persist across reps, slots rotate)
        xs = stack.enter_context(tc.tile_pool(name="xs", bufs=1))
        qst_pool = stack.enter_context(tc.tile_pool(name="qst",
                                                    bufs=qst_bufs))
        vstg = stack.enter_context(tc.tile_pool(name="vstg", bufs=4))
        mxw = stack.enter_context(tc.tile_pool(name="mxw", bufs=2))
        scw = stack.enter_context(tc.tile_pool(name="scw", bufs=pe_bufs))
        yw = stack.enter_context(tc.tile_pool(name="yw", bufs=5))
        # PSUM pools: 4 + 2 + 2 banks
        big2 = stack.enter_context(tc.tile_pool(name="big2", bufs=big_bufs,
                                                space="PSUM"))
        po2 = stack.enter_context(tc.tile_pool(name="po2", bufs=2,
                                               space="PSUM"))
        mps2 = stack.enter_context(tc.tile_pool(name="mps2", bufs=2,
                                                space="PSUM"))

        # --- weights + constants: SBUF tiles ---
        ident = cpool.tile([P, P], f32, name="ident")
        ident16 = cpool.tile([P, P], f16, name="ident16")
        tri_std = cpool.tile([P, P], f16, name="tri_std")
        tri_t = cpool.tile([P, P], f16, name="tri_t")   # [k,q]: NEG16 if k>q
        sel = cpool.tile([16, 16 * DH], f16, name="sel")
        wq_hi = wpool.tile([P, KSUB, CB], f16, name="wq_hi")
        wq8_t = wpool.tile([P, KSUB, 2, CB], f8e5, name="wq8")
        wk_hi = wpool.tile([P, KSUB, CB], f16, name="wk_hi")
        wk8_t = wpool.tile([P, KSUB, 2, CB], f8e5, name="wk8")
        wv_t = wpool.tile([P, KSUB, CB], f16, name="wv_t")
        wo_t = wpool.tile([P, 2, NS], f16, name="wo_t")
        sbq_t = wpool.tile([P, 2], f32, name="sbq_t")

        # --- persistent activations ---
        # q65[h]: rows 0:64 = qA (f16 hi of scaled q), row 64 = m (row max)
        # k65[h]: rows 0:64 = kA, row 64 = -1  => hi matmul yields kA.qA - m
        # q8/k8 (head-pair packed, rows h%2*64): DoubleRow fp8 correction
        #   q8 slots: (qB*64, qA/64); k8 slots: (kA/64, kB*64)
        #   => DR(k8, q8) = kA.qB + kB.qA (the f16-rounding correction)
        q65 = [qkpool.tile([65, S], f16, name=f"q65{h}") for h in range(HPC)]
        k65 = [qkpool.tile([65, S], f16, name=f"k65{h}") for h in range(HPC)]
        q8 = [qkpool.tile([P, 2, S], f8, name=f"q8{hp}") for hp in range(2)]
        k8 = [qkpool.tile([P, 2, S], f8, name=f"k8{hp}") for hp in range(2)]
        vt = [qkpool.tile([P, NQB, DH + 1], f16, name=f"v{h}")
              for h in range(HPC)]
        oU = qkpool.tile([P, 2, S], f16, name="oU")     # o.T (norm in place)
        sums = qkpool.tile([16, 512], f32, name="sums")
        rc = qkpool.tile([16, 512], f16, name="rc")
        mh = [qkpool.tile([P, NQB], f32, name=f"m{h}") for h in range(HPC)]

        xh_v = xh_d.rearrange("(ko p) s -> p ko s", p=P)
        xp8_v = xp8_d.rearrange("(ko p) two s -> p ko two s", p=P)

        # --- on-chip init (gpsimd/iota; no DMA) ---
        make_identity(nc, ident[:])
        make_identity(nc, ident16[:])
        # f16 causal masks (NEG16 = -60000 is f16-representable and large
        # enough: scores are at most a few thousand in magnitude)
        make_causal_mask(nc, tri_std[:], mask_val=NEG16)  # [q,k]
        nc.gpsimd.memset(tri_t[:], 0.0)
        nc.gpsimd.affine_select(
            out=tri_t[:], in_=tri_t[:],
            compare_op=mybir.AluOpType.is_ge, fill=NEG16, base=0,
            # keep where -x + y >= 0 i.e. q >= k; fill where k > q
            pattern=[[1, P]], channel_multiplier=-1)
        nc.gpsimd.memset(sums[:], 1.0)
        for h in range(HPC):
            nc.gpsimd.memset(k65[h][64:65, :], -1.0)
            nc.gpsimd.memset(vt[h][:, :, DH:DH + 1], 1.0)

        # alternate small PSUM tiles over two pools => 4-deep buffering
        _rr = [0]

        def small_ps(name):
            _rr[0] ^= 1
            pool = (mps2, po2)[_rr[0]]
            return pool.tile([P, 512], f32, name=name,
                             tag=("mps", "po")[_rr[0]])

        def max_chunk(h, qbs, wide=False):
            """Causal row maxima matmuls+reductions for head h, q blocks qbs.

            wide=True routes [128,1024] tiles through big2 (2 banks each):
            2x fewer DVE reductions and 4 matmuls of runway per tile pair.
            """
            for qb in qbs:
                valid_all = qb * P + P
                tile_w = 1024 if wide else 512
                nt = (valid_all + tile_w - 1) // tile_w
                for ti in range(nt):
                    base = ti * tile_w
                    w = min(tile_w, valid_all - base)
                    if wide:
                        ps = big2.tile([P, 1024], f32, name="mwps",
                                       tag="big")
                    else:
                        ps = small_ps("mps")
                    last = ti == nt - 1
                    for sub in range(0, w, 512):
                        sw = min(512, w - sub)
                        diag_here = last and sub + sw == w
                        nc.tensor.matmul(
                            ps[:, sub:sub + sw],
                            q65[h][0:64, qb * P:(qb + 1) * P],
                            k65[h][0:64, base + sub:base + sub + sw],
                            start=True, stop=not diag_here)
                        if diag_here:
                            # diagonal causal mask via PE tri accumulate
                            nc.tensor.matmul(
                                ps[:, w - P:w], ident16[:], tri_std[:],
                                start=False, stop=True,
                                skip_group_check=True)
                    if ti == 0:
                        nc.vector.tensor_reduce(
                            mh[h][:, qb:qb + 1], ps[:, 0:w],
                            axis=mybir.AxisListType.X,
                            op=mybir.AluOpType.max)
                    else:
                        tm = mxw.tile([P, 1], f32, name="tm")
                        nc.vector.tensor_reduce(
                            tm[:], ps[:, 0:w],
                            axis=mybir.AxisListType.X,
                            op=mybir.AluOpType.max)
                        nc.vector.tensor_tensor(
                            mh[h][:, qb:qb + 1], mh[h][:, qb:qb + 1],
                            tm[:], mybir.AluOpType.max)

        def maxb_partA(h, qbs=range(8, NQB)):
            # k[0:1024] portion of back q blocks: only needs s-chunk-0 k and
            # s-chunk-1 q, so it runs ~20us before the k s1 projection lands
            for qb in qbs:
                for sub in range(2):
                    ps = small_ps("mps")
                    nc.tensor.matmul(
                        ps[:, 0:512],
                        q65[h][0:64, qb * P:(qb + 1) * P],
                        k65[h][0:64, sub * 512:sub * 512 + 512],
                        start=True, stop=True)
                    if sub == 0:
                        nc.vector.tensor_reduce(
                            mh[h][:, qb:qb + 1], ps[:, 0:512],
                            axis=mybir.AxisListType.X,
                            op=mybir.AluOpType.max)
                    else:
                        tm = mxw.tile([P, 1], f32, name="tm")
                        nc.vector.tensor_reduce(
                            tm[:], ps[:, 0:512],
                            axis=mybir.AxisListType.X,
                            op=mybir.AluOpType.max)
                        nc.vector.tensor_tensor(
                            mh[h][:, qb:qb + 1], mh[h][:, qb:qb + 1],
                            tm[:], mybir.AluOpType.max)

        def maxb_partB(h, qbs):
            # k[1024:qb*128+128] remainder (diagonal tri mask included)
            for qb in qbs:
                valid = qb * P + P
                w = valid - 1024
                ps = big2.tile([P, 1024], f32, name="mwps", tag="big")
                for sub in range(0, w, 512):
                    sw = min(512, w - sub)
                    diag_here = sub + sw == w
                    nc.tensor.matmul(
                        ps[:, sub:sub + sw],
                        q65[h][0:64, qb * P:(qb + 1) * P],
                        k65[h][0:64, 1024 + sub:1024 + sub + sw],
                        start=True, stop=not diag_here)
                    if diag_here:
                        nc.tensor.matmul(
                            ps[:, w - P:w], ident16[:], tri_std[:],
                            start=False, stop=True, skip_group_check=True)
                tm = mxw.tile([P, 1], f32, name="tm")
                nc.vector.tensor_reduce(
                    tm[:], ps[:, 0:w], axis=mybir.AxisListType.X,
                    op=mybir.AluOpType.max)
                nc.vector.tensor_tensor(
                    mh[h][:, qb:qb + 1], mh[h][:, qb:qb + 1],
                    tm[:], mybir.AluOpType.max)

        def max_finish(h):
            tpm = mps2.tile([NQB, P], f32, name="tpm", tag="mps")
            nc.tensor.transpose(tpm[:], mh[h][:, 0:NQB], ident[:])
            mt = mxw.tile([NQB, P], f16, name="mt")
            nc.scalar.copy(mt[:], tpm[:])
            # SP-queue DMA: skips the Pool staging backlog at the
            # phase-1 -> phase-2 transition
            nc.sync.dma_start(q65[h][64:65, :], mt[:])

        for _rep in range(repeat):
            first = _rep == 0
            # ---- x DMAs (+ weight DMAs threaded in priority order, rep 0).
            # cs0 weight halves lead so the first projection starts early;
            # cs1 halves + wv/wo trail the sc1 x pieces. ----
            wv_q = wqh_d.rearrange("(ko p) c -> p ko c", p=P)
            wv_q8 = wq8_d.rearrange("(ko p) two c -> p ko two c", p=P)
            wv_k = wkh_d.rearrange("(ko p) c -> p ko c", p=P)
            wv_k8 = wk8_d.rearrange("(ko p) two c -> p ko two c", p=P)
            if first:
                nc.sync.dma_start(wq_hi[:, :, 0:P], wv_q[:, :, 0:P])
            xhs = [[None] * 4, [None] * 4]
            xls = [[None] * 4, [None] * 4]
            for sc in range(2):
                ss = slice(sc * 1024, (sc + 1) * 1024)
                for piece in range(4):
                    t = xs.tile([P, KSUB // 4, 1024], f16,
                                name=f"xh{piece}{sc}")
                    nc.sync.dma_start(
                        t[:], xh_v[:, piece * 2:piece * 2 + 2, ss])
                    xhs[sc][piece] = t
                    if first and sc == 0 and piece == 0:
                        nc.sync.dma_start(wq8_t[:], wv_q8)
                for piece in range(4):
                    t = xs.tile([P, KSUB // 4, 2, 1024], f8e5,
                                name=f"xp{piece}{sc}")
                    for sl in range(2):
                        nc.sync.dma_start(
                            t[:, :, sl, :],
                            xp8_v[:, piece * 2:piece * 2 + 2, sl, ss])
                    xls[sc][piece] = t
                if first and sc == 0:
                    nc.sync.dma_start(
                        sbq_t[:], sbq_d.rearrange("(cs p) -> p cs", p=P))
                    nc.sync.dma_start(wk_hi[:, :, 0:P], wv_k[:, :, 0:P])
                    nc.sync.dma_start(wk8_t[:], wv_k8)
            if first:
                nc.sync.dma_start(wq_hi[:, :, P:CB], wv_q[:, :, P:CB])
                nc.sync.dma_start(wk_hi[:, :, P:CB], wv_k[:, :, P:CB])
                nc.sync.dma_start(wv_t[:],
                                  wv_d.rearrange("(ko p) c -> p ko c", p=P))
                nc.sync.dma_start(wo_t[:],
                                  wo_d.rearrange("(cs p) j -> p cs j", p=P))
                nc.sync.dma_start(sel[:], sel_d[:])

            # ====== Phase 1: QKV projections + interleaved maxima ======
            def proj_qk(cs, proj, sc, filler=None):
                csl = slice(cs * P, (cs + 1) * P)
                w_hi = wq_hi if proj == "q" else wk_hi
                w_8 = wq8_t if proj == "q" else wk8_t
                xh_c, xp_c = xhs[sc], xls[sc]
                ss = slice(sc * 1024, (sc + 1) * 1024)
                ps = big2.tile([P, 1024], f32, name="qkps", tag="big")
                # pass 1: f16 hi; pass 2: fp8-e5m2 DoubleRow correction
                # ((wh/84)(xl*84) + (wl*117)(xh/117)) at 0.5 cycles/col
                for ko in range(KSUB):
                    for half in range(2):
                        hsl = slice(half * 512, (half + 1) * 512)
                        nc.tensor.matmul(
                            ps[:, hsl],
                            w_hi[:, ko, csl],
                            xh_c[ko // 2][:, ko % 2, hsl],
                            start=(ko == 0), stop=False)
                    if filler:
                        filler.pop(0)()
                for ko in range(KSUB):
                    for half in range(2):
                        hsl = slice(half * 512, (half + 1) * 512)
                        nc.tensor.matmul(
                            ps[:, hsl],
                            w_8[:, ko, :, csl],
                            xp_c[ko // 2][:, ko % 2, :, hsl],
                            start=False,
                            stop=(ko == KSUB - 1 and half == 1),
                            perf_mode=DR)
                    if filler:
                        filler.pop(0)()
                if proj == "q":
                    nc.scalar.activation(ps[:], ps[:], FX.Identity,
                                         bias=sbq_t[:, cs:cs + 1])
                # staging: Act hi-copy + f32 snapshot; Pool residual
                # subtract (f16) then fp8 slot conversions for DoubleRow
                At = q65 if proj == "q" else k65
                T8 = q8[cs] if proj == "q" else k8[cs]
                qBt = qst_pool.tile([P, 1024], f16, name="qBt")
                subs = []
                for hh in range(2):
                    h = 2 * cs + hh
                    rsl = slice(hh * 64, (hh + 1) * 64)
                    srcp = ps[rsl, :]
                    st32 = qst_pool.tile([64, 1024], f32, name="st32")
                    nc.scalar.copy(At[h][0:64, ss], srcp)
                    nc.scalar.copy(st32[:], srcp)
                    subs.append((qBt[rsl, :], st32, At[h][0:64, ss]))
                for out_ap, st32, hi_ap in subs:
                    nc.gpsimd.tensor_tensor(out_ap, st32[:], hi_ap,
                                            mybir.AluOpType.subtract)
                # fp8 slots: lo slot is the residual*64, hi slot is hi/64
                lo_slot, hi_slot = (0, 1) if proj == "q" else (1, 0)
                nc.gpsimd.tensor_scalar_mul(T8[:, lo_slot, ss], qBt[:], 64.0)
                for hh in range(2):
                    h = 2 * cs + hh
                    rsl = slice(hh * 64, (hh + 1) * 64)
                    nc.gpsimd.tensor_scalar_mul(T8[rsl, hi_slot, ss],
                                                At[h][0:64, ss], 1.0 / 64.0)

            QF = range(0, 8)      # front q blocks (need only s-chunk 0)

            def proj_v(st, dve_stage=False):
                sc, sti = divmod(st, 8)
                psv = mps2.tile([P, 512], f32, name="vps", tag="mps")
                for ko in range(KSUB):
                    nc.tensor.matmul(
                        psv[:, 0:CB],
                        xhs[sc][ko // 2][:, ko % 2, sti * P:(sti + 1) * P],
                        wv_t[:, ko, :],
                        start=(ko == 0), stop=(ko == KSUB - 1))
                vsg = vstg.tile([P, CB], f16, name="vsg")
                # as a phase-2 filler, stage on DVE so the Act queue stays
                # clear for the exp chain
                if dve_stage:
                    nc.vector.tensor_copy(vsg[:], psv[:, 0:CB])
                else:
                    nc.scalar.copy(vsg[:], psv[:, 0:CB])
                for h in range(HPC):
                    nc.gpsimd.tensor_copy(
                        vt[h][:, st, 0:DH], vsg[:, h * DH:(h + 1) * DH])

            # Phase 1. DVE queue order is the critical resource: h0/h1
            # maxima must complete before h2/h3 reductions enqueue, and
            # every DVE-heavy stretch is interleaved with PE-only work
            # (projection matmuls, v tiles).
            mx = "maxmm" not in ablate
            pa = {h: [lambda h=h, qb=qb: maxb_partA(h, (qb,))
                      for qb in range(8, NQB)] for h in range(HPC)} \
                if mx else {h: [] for h in range(HPC)}
            proj_qk(0, "q", 0)
            proj_qk(0, "k", 0)
            if mx:
                max_chunk(0, QF)
                max_chunk(1, QF)
            proj_qk(0, "q", 1)
            proj_qk(0, "k", 1, filler=pa[0])
            if mx:
                for i, qb0 in enumerate(range(8, NQB, 2)):
                    maxb_partB(0, (qb0, qb0 + 1))
                    proj_v(i)
                max_finish(0)
            proj_qk(1, "q", 0, filler=pa[1][:4])
            proj_qk(1, "k", 0, filler=pa[1][4:])
            if mx:
                for i, qb0 in enumerate(range(8, NQB, 2)):
                    maxb_partB(1, (qb0, qb0 + 1))
                    proj_v(4 + i)
                max_finish(1)
                max_chunk(2, QF)
                max_chunk(3, QF)
            else:
                for st in range(8):
                    proj_v(st)
            # pa[2]/pa[3] read q65 s-chunk 1, so they may only start after
            # the (1,"q",1) blob's staging — both ride the (1,"k",1) blob
            proj_qk(1, "q", 1)
            proj_qk(1, "k", 1, filler=pa[2] + pa[3])

            def maxb_units(h, qb0s):
                return [lambda h=h, qb0=qb0: maxb_partB(
                    h, (qb0, qb0 + 1)) for qb0 in qb0s]

            def v_units(sts):
                return [lambda st=st: proj_v(st, dve_stage=True)
                        for st in sts]

            def _maxA_wide(h, qb):
                # big2-based partA variant, safe as a score_head filler
                # (po2/mps2 untouched)
                ps = big2.tile([P, 1024], f32, name="mwps", tag="big")
                for sub in range(2):
                    nc.tensor.matmul(
                        ps[:, sub * 512:sub * 512 + 512],
                        q65[h][0:64, qb * P:(qb + 1) * P],
                        k65[h][0:64, sub * 512:sub * 512 + 512],
                        start=True, stop=True)
                nc.vector.tensor_reduce(
                    mh[h][:, qb:qb + 1], ps[:],
                    axis=mybir.AxisListType.X, op=mybir.AluOpType.max)

            def maxA_units(h):
                return [lambda h=h, qb=qb: _maxA_wide(h, qb)
                        for qb in range(8, NQB)]

            # ====== Phase 2: scores / exp / pv / norm / proj ======
            def score_head(qc, h, filler=None, fstride=1):
                nkb = (8 * qc + 8) if "scores" not in ablate else 0
                po = [po2.tile([DH + 1, 512], f32,
                               name=f"po{half}", tag="po")
                      for half in range(2)]
                def emit_pv(kb, off, pe):
                    for half in range(2):
                        lo = max(off, half * 512)
                        hi = (half + 1) * 512
                        if lo >= hi:
                            continue
                        stop_kb = (8 * qc + 3) if half == 0 else (nkb - 1)
                        nc.tensor.matmul(
                            po[half][:, lo - half * 512:512],
                            vt[h][:, kb, :], pe[:, lo:hi],
                            start=(kb == 0), stop=(kb == stop_kb))

                # pv is emitted two kb behind scores+exp so the in-order PE
                # queue never parks on an exp: scores(kb+1..2) run while
                # exp(kb) is in flight, then pv(kb) is ready.
                from collections import deque as _dq
                pending_pv = _dq()
                for kb in range(nkb):
                    j = max(0, kb - 8 * qc)
                    off = j * P
                    ks = slice(kb * P, (kb + 1) * P)
                    ps = big2.tile([P, 1024], f32, name="sps", tag="big")
                    qbase = qc * 1024
                    regions = [(max(off, r0), r0 + 512,
                                (kb >= 8 * qc) and (r0 <= off < r0 + 512))
                               for r0 in range(off - off % 512, 1024, 512)]
                    hp, hr = h // 2, (h % 2) * 64
                    for lo, r1, _ in regions:
                        # 65-contraction: kA.qA - m (the -1/m rows ride)
                        nc.tensor.matmul(
                            ps[:, lo:r1], k65[h][:, ks],
                            q65[h][:, qbase + lo:qbase + r1],
                            start=True, stop=False)
                    for lo, r1, diag_here in regions:
                        # fp8 DoubleRow correction: kA.qB + kB.qA
                        for c0 in range(lo, r1, 256):
                            c1 = min(c0 + 256, r1)
                            nc.tensor.matmul(
                                ps[:, c0:c1],
                                k8[hp][hr:hr + 64, :, ks],
                                q8[hp][hr:hr + 64, :,
                                       qbase + c0:qbase + c1],
                                start=False,
                                stop=(c1 == r1 and not diag_here),
                                perf_mode=DR)
                        if diag_here:
                            nc.tensor.matmul(
                                ps[:, off:off + P], ident16[:], tri_t[:],
                                start=False, stop=True,
                                skip_group_check=True)
                    if "exp" in ablate:
                        continue
                    pe = scw.tile([P, 1024], f16, name="pe")
                    nc.scalar.activation(pe[:, off:1024],
                                         ps[:, off:1024], FX.Exp)
                    if "pv" in ablate:
                        continue
                    if len(pending_pv) >= 2:
                        emit_pv(*pending_pv.popleft())
                    if filler and kb % fstride == 0:
                        filler.pop(0)()
                        if len(filler) > nkb - kb - 1:
                            filler.pop(0)()
                    pending_pv.append((kb, off, pe))
                if "pv" not in ablate and "exp" not in ablate:
                    while pending_pv:
                        emit_pv(*pending_pv.popleft())
                if "norm" in ablate:
                    return
                # stash unnormalized o and sums (DVE, PSUM reads)
                for half in range(2):
                    q5 = slice((2 * qc + half) * 512,
                               (2 * qc + half + 1) * 512)
                    nc.vector.tensor_copy(
                        oU[(h % 2) * DH:(h % 2 + 1) * DH,
                           h // 2, q5], po[half][0:DH, :])
                    i5 = h * 4 + 2 * qc + half
                    smt = mxw.tile([1, 512], f32, name="smt")
                    nc.vector.tensor_copy(smt[:], po[half][DH:DH + 1, :])
                    nc.gpsimd.dma_start(sums[i5:i5 + 1, :], smt[:])

            def norm_units(qc, halves=(0, 1)):
                units = []

                def recip_unit():
                    with nc.allow_low_precision(
                            reason="1/sums to f16: rel err ~5e-4 "
                                   "well inside tolerance"):
                        nc.vector.reciprocal(rc[:], sums[:])
                units.append(recip_unit)
                for half in halves:
                    for h in range(HPC):
                        def mult_unit(h=h, half=half):
                            i = h * 4 + 2 * qc + half
                            b0 = (h % 2) * DH
                            rbp = mps2.tile([P, 512], f32, name="rbp",
                                            tag="mps")
                            nc.tensor.matmul(rbp[b0:b0 + DH, :],
                                             sel[:, i * DH:(i + 1) * DH],
                                             rc[:], start=True, stop=True)
                            qsl = slice((2 * qc + half) * 512,
                                        (2 * qc + half + 1) * 512)
                            nc.vector.tensor_tensor(
                                oU[b0:b0 + DH, h // 2, qsl],
                                oU[b0:b0 + DH, h // 2, qsl],
                                rbp[b0:b0 + DH, :], mybir.AluOpType.mult)
                        units.append(mult_unit)
                return units

            def proj_units(qc, sts, act_yt=False):
                units = []
                for st in sts:
                    for jc in range(2):
                        def unit(st=st, jc=jc):
                            jsl = slice(jc * 512, (jc + 1) * 512)
                            if qc == 1:
                                ps = small_ps("yp")
                            else:
                                ps = mps2.tile([P, 512], f32, name="yp",
                                               tag="mps")
                            for cs2 in range(2):
                                nc.tensor.matmul(
                                    ps[:], oU[:, cs2, st * P:(st + 1) * P],
                                    wo_t[:, cs2, jsl],
                                    start=(cs2 == 0), stop=(cs2 == 1))
                            yt = yw.tile([P, 512], f16, name="yt")
                            # once exps are done, Act drains the psum reads
                            if act_yt:
                                nc.scalar.copy(yt[:], ps[:])
                            else:
                                nc.vector.tensor_copy(yt[:], ps[:])
                            nc.sync.dma_start(y_v[:, st, jsl], yt[:])
                        units.append(unit)
                return units

            # ---- Phase B: h0/h1 scores+pv overlapped with the cs1
            # projections, the whole h2/h3 max pass, and v st8..15 ----
            mx = "maxmm" not in ablate
            score_head(0, 0)
            score_head(0, 1)
            score_head(1, 0, filler=(maxb_units(2, range(8, NQB, 2))
                                     + v_units(range(8, NQB))) if mx
                       else None)
            if mx:
                max_finish(2)
            score_head(1, 1, filler=maxb_units(3, range(8, NQB, 2)) if mx
                       else None)
            if mx:
                max_finish(3)
            # ---- Phase C: h2/h3 scores+pv, norm/proj q-group 0 fillers ----
            score_head(0, 2)
            score_head(1, 2)
            score_head(0, HPC - 1)
            # normalize+project q-group 0 entirely as fillers inside the
            # last q1 head (dependency-safe: units consumed in order)
            p0_units = (norm_units(0, halves=(0,))
                        + proj_units(0, range(4))
                        + norm_units(0, halves=(1,))[1:]
                        + proj_units(0, range(4, 8))
                        if "norm" not in ablate
                        and "proj" not in ablate else [])
            score_head(1, HPC - 1, filler=p0_units)
            for u in p0_units:
                u()
            if "norm" not in ablate:
                # half-split tail: projection of each q-half starts as soon
                # as that half's normalization lands
                for u in norm_units(1, halves=(0,)):
                    u()
                for u in proj_units(1, range(8, 12), act_yt=True):
                    u()
                for u in norm_units(1, halves=(1,))[1:]:
                    u()
                for u in proj_units(1, range(12, 16), act_yt=True):
                    u()
            if debug_dump:
                nc.gpsimd.dma_start(dbg_oU[:],
                                    oU[:].rearrange("p a b -> p (a b)"))
                nc.sync.dma_start(dbg_sums[:], sums[:])
                for h in range(HPC):
                    nc.sync.dma_start(dbg_mh[:, h * NQB:(h + 1) * NQB],
                                      mh[h][:])

    nc.compile()
    return nc


def _split16(a):
    hi = a.astype(np.float16)
    lo = (a - hi.astype(np.float32)).astype(np.float16)
    return hi, lo


def _pack8(lo_scaled, hi_scaled):
    import ml_dtypes
    return np.ascontiguousarray(np.stack(
        [np.asarray(lo_scaled, ml_dtypes.float8_e5m2),
         np.asarray(hi_scaled, ml_dtypes.float8_e5m2)], axis=1))


def _prep_core(c, x, Wq, bq, Wk, Wv, Wo):
    b, g = divmod(c, 4)
    cols = slice(g * CB, (g + 1) * CB)
    xT = np.ascontiguousarray(x[b].T).astype(np.float32)
    xh, xl = _split16(xT)
    xl = xl.astype(np.float32)
    # DoubleRow pair: (wh/84)(xl*84) + (wl*117)(xh/117)
    xp8 = _pack8(xl * 84.0, xh.astype(np.float32) / 117.0)
    wq = (SCALE * Wq[cols]).T.astype(np.float32)
    wqh, wql = _split16(wq)
    wq8 = _pack8(wqh.astype(np.float32) / 84.0, wql.astype(np.float32) * 117.0)
    wk = (SCALE * Wk[cols]).T.astype(np.float32)
    wkh, wkl = _split16(wk)
    wk8 = _pack8(wkh.astype(np.float32) / 84.0, wkl.astype(np.float32) * 117.0)
    wv = Wv[cols].T.astype(np.float16)
    wo = np.ascontiguousarray(Wo[:, cols].T).astype(np.float16)
    sbq = (SCALE * bq[cols]).astype(np.float32)
    sel = np.zeros((16, 16 * DH), np.float16)
    for i in range(16):
        sel[i, i * DH:(i + 1) * DH] = 1.0
    return {"xh": xh, "xp8": xp8, "wqh": wqh, "wq8": wq8, "wkh": wkh,
            "wk8": wk8, "wv": np.ascontiguousarray(wv),
            "wo": wo, "sbq": sbq, "sel": sel}


def kernel(x, mask, Wq, bq, Wk, Wv, bv, Wo, bo):
    global _PROG
    if _PROG is None:
        _PROG = build_program()
    x = np.asarray(x, dtype=np.float32)
    in_maps = [_prep_core(c, x, np.asarray(Wq), np.asarray(bq),
                          np.asarray(Wk), np.asarray(Wv), np.asarray(Wo))
               for c in range(8)]
    res = bass_utils.run_bass_kernel_spmd(_PROG, in_maps,
                                          core_ids=list(range(8)))
    host_bias = (np.asarray(bv, np.float32) @ np.asarray(Wo, np.float32).T
                 + np.asarray(bo, np.float32))
    out = np.empty((B, S, NS), np.float32)
    for b in range(B):
        acc = res.results[4 * b]["y"].astype(np.float32)
        for g in range(1, 4):
            acc += res.results[4 * b + g]["y"].astype(np.float32)
        out[b] = acc + host_bias
    return out

